# revision 7
# baseline (speedup 1.0000x reference)
"""Trainium2 kernel for MagFace/AdaCos-style margin softmax-CE loss.

Strategy (8 cores, class-parallel, fp8 DoubleRow):
  - Host pre-normalizes x and W rows (fp32), scales by 16, quantizes to
    fp8 e4m3, and pre-transposes both into [128d, k, cols] layouts.
    Classes are sharded across 8 cores (12500 each, padded to 12544).
  - Device per core: for each class-group (6 x 2048 + 1 x 256 cols),
    fp8 DoubleRow matmuls (256-deep contraction, 0.5 cyc/col) compute
    256*cos into a 4-bank PSUM mega-tile per b-tile; one giant ScalarE
    Exp instruction (scale=S/256) produces exp(S*cos) in bf16 and the
    per-sample class-sum via accum_out; DVE reduce_max tracks the
    per-sample max of exp(S*cos).
  - Pad classes have zero weight -> cos 0 -> exp contributes exactly
    1.0, subtracted on host.
  - Host does all the label-column margin math (phi, loss_g) in fp64,
    corrects the sum-exp for the label column (using the quantized
    label cosine so the correction matches the device's own term),
    and combines: CE + lambda_g * g-loss and top-1 accuracy.
"""

import math
import sys

sys.path.insert(0, "/opt/trn_rl_repo")
sys.path.insert(0, "/opt/trn_rl_repo/concourse")

import numpy as np

# ---- problem constants ----
B = 512
D = 512
C = 100000
NCORES = 8
C_SH = C // NCORES          # 12500
C_PAD = 12544               # 24.5 chunks of 512
N_PAD = C_PAD - C_SH        # 44 zero-pad classes per core
S = 30.0
N_U = 110.0
N_L = 10.0
M_U = 1.0
M_L = 0.1
LAMBDA_G = 35.0
QSCALE = 16.0               # fp8 quantization scale for unit-norm rows
PSUM_SCALE = QSCALE * QSCALE  # matmul result = PSUM_SCALE * cos

# class-group column widths per b-tile: 6 * 2048 + 256 = 12544
GROUP_W = [2048] * 6 + [256]
NGROUPS = len(GROUP_W)
NB = 4                      # b-tiles of 128 samples

_cache = {}


def _emit_body(nc, tc, tensors, mybir, bass):
    F32 = mybir.dt.float32
    BF16 = mybir.dt.bfloat16
    FP8 = mybir.dt.float8e4
    ACT = mybir.ActivationFunctionType
    ALU = mybir.AluOpType
    PM = mybir.MatmulPerfMode

    xq_dram = tensors["xq"]
    wq_dram = tensors["wq"]
    sums_dram = tensors["sums"]
    maxacc_dram = tensors["maxacc"]
    wq_ap = wq_dram.ap()

    with (
        tc.tile_pool(name="persist", bufs=1) as pp,
        tc.tile_pool(name="maxp", bufs=2) as max_pool,
        tc.tile_pool(name="expp", bufs=5) as exp_pool,
        tc.tile_pool(name="psum", bufs=2, space=bass.MemorySpace.PSUM) as psum_pool,
    ):
        xq_sb = pp.tile([128, 4, B], FP8)
        nc.sync.dma_start(xq_sb[:], xq_dram.ap())

        # early activation-table load: dummy exp on a memset tile
        warm_sb = pp.tile([128, 8], F32)
        nc.gpsimd.memset(warm_sb[:], 0.0)
        warm_out = pp.tile([128, 8], BF16)
        nc.scalar.activation(warm_out[:], warm_sb[:], ACT.Exp)

        # whole weight shard stays resident in SBUF (49 KiB/partition)
        wq_sb = pp.tile([128, 4, C_PAD], FP8)
        DMA_W = 1024
        for c0 in range(0, C_PAD, DMA_W):
            c1 = min(c0 + DMA_W, C_PAD)
            nc.sync.dma_start(wq_sb[:, :, c0:c1], wq_ap[:, :, c0:c1])

        sums_sb = pp.tile([128, NGROUPS * NB], F32)

        # PE warm-up: junk matmuls on xq to lift the HAM clock gate while
        # the first weight chunks stream in
        warm_ps = psum_pool.tile([128, 2048], F32, tag="ps")
        for r in range(20):
            nc.tensor.matmul(
                warm_ps[:, :512],
                xq_sb[:, 0:2, 0:128],
                xq_sb[:, 0:2, 0:512],
                start=True, stop=True,
                perf_mode=PM.DoubleRow,
                skip_group_check=True,
            )

        for bt in range(NB):
            maxacc = max_pool.tile([128, 2048], BF16, tag="maxacc")
            col = 0
            for g in range(NGROUPS):
                W = GROUP_W[g]
                ps = psum_pool.tile([128, 2048], F32, tag="ps")
                for kp in range(2):
                    lhsT = xq_sb[:, 2 * kp : 2 * kp + 2, bt * 128 : (bt + 1) * 128]
                    nj = max(W // 512, 1)
                    for j in range(nj):
                        w0 = j * 512
                        w1 = min(w0 + 512, W)
                        nc.tensor.matmul(
                            ps[:, w0:w1],
                            lhsT,
                            wq_sb[:, 2 * kp : 2 * kp + 2, col + w0 : col + w1],
                            start=(kp == 0),
                            stop=(kp == 1),
                            perf_mode=PM.DoubleRow,
                            skip_group_check=True,
                        )
                idx = bt * NGROUPS + g
                exp_t = exp_pool.tile([128, 2048], BF16, tag="exp")
                nc.scalar.activation(
                    exp_t[:, :W], ps[:, :W], ACT.Exp,
                    scale=float(S / PSUM_SCALE),
                    accum_out=sums_sb[:, idx : idx + 1],
                )
                if g == 0:
                    nc.vector.tensor_copy(maxacc[:], exp_t[:])
                else:
                    nc.vector.tensor_tensor(
                        out=maxacc[:, :W], in0=maxacc[:, :W],
                        in1=exp_t[:, :W], op=ALU.max,
                    )
                col += W
            nc.sync.dma_start(
                maxacc_dram.ap()[:, bt, :], maxacc[:]
            )

        nc.sync.dma_start(sums_dram.ap(), sums_sb[:])


def _build(repeat=1):
    from concourse import bass, bacc, tile, mybir

    F32 = mybir.dt.float32
    FP8 = mybir.dt.float8e4

    nc = bacc.Bacc("TRN2", target_bir_lowering=False, debug=False)

    tensors = {
        "xq": nc.dram_tensor("xq", [128, 4, B], FP8, kind="ExternalInput"),
        "wq": nc.dram_tensor("wq", [128, 4, C_PAD], FP8, kind="ExternalInput"),
        "sums": nc.dram_tensor(
            "sums", [128, NGROUPS * NB], F32, kind="ExternalOutput"
        ),
        "maxacc": nc.dram_tensor(
            "maxacc", [128, NB, 2048], mybir.dt.bfloat16, kind="ExternalOutput"
        ),
    }

    with tile.TileContext(nc) as tc:
        for _ in range(repeat):
            _emit_body(nc, tc, tensors, mybir, bass)

    nc.compile()
    return nc


class Runner:
    """Persistent jitted 8-core runner (inputs stay device-resident)."""

    def __init__(self, repeat=1):
        import jax
        from jax.sharding import Mesh, PartitionSpec, NamedSharding
        from jax.experimental.shard_map import shard_map
        from concourse import bass2jax, mybir

        self.jax = jax
        nc = _build(repeat)
        self.nc = nc
        bass2jax.install_neuronx_cc_hook()

        partition_name = (
            nc.partition_id_tensor.name if nc.partition_id_tensor else None
        )
        in_names, out_names, out_avals, zero_shapes = [], [], [], []
        for alloc in nc.m.functions[0].allocations:
            if not isinstance(alloc, mybir.MemoryLocationSet):
                continue
            name = alloc.memorylocations[0].name
            if alloc.kind == "ExternalInput":
                if name == partition_name:
                    continue
                in_names.append(name)
            elif alloc.kind == "ExternalOutput":
                shape = tuple(alloc.tensor_shape)
                dtype = mybir.dt.np(alloc.dtype)
                out_names.append(name)
                out_avals.append(jax.core.ShapedArray(shape, dtype))
                zero_shapes.append((shape, dtype))
        self.in_names = in_names
        self.out_names = out_names
        self.out_avals = out_avals
        self.zero_shapes = zero_shapes
        n_params = len(in_names)
        n_outs = len(out_names)
        all_in_names = in_names + out_names
        if partition_name is not None:
            all_in_names = all_in_names + [partition_name]

        def _body(*args):
            operands = list(args)
            if partition_name is not None:
                operands.append(bass2jax.partition_id_tensor())
            outs = bass2jax._bass_exec_p.bind(
                *operands,
                out_avals=tuple(out_avals),
                in_names=tuple(all_in_names),
                out_names=tuple(out_names),
                lowering_input_output_aliases=(),
                sim_require_finite=True,
                sim_require_nnan=True,
                nc=nc,
            )
            return tuple(outs)

        devices = jax.devices()[:NCORES]
        self.mesh = Mesh(np.asarray(devices), ("core",))
        in_specs = (PartitionSpec("core"),) * (n_params + n_outs)
        out_specs = (PartitionSpec("core"),) * n_outs
        self.sharding = NamedSharding(self.mesh, PartitionSpec("core"))
        self.fn = jax.jit(
            shard_map(
                _body, mesh=self.mesh, in_specs=in_specs, out_specs=out_specs,
                check_rep=False,
            ),
            donate_argnums=tuple(range(n_params, n_params + n_outs)),
            keep_unused=True,
        )

    def put_inputs(self, in_maps):
        jax = self.jax
        concat = [
            np.concatenate([np.asarray(m[name]) for m in in_maps], axis=0)
            for name in self.in_names
        ]
        return [jax.device_put(a, self.sharding) for a in concat]

    def zeros(self):
        jax = self.jax
        return [
            jax.device_put(np.zeros((NCORES * s[0], *s[1:]), d), self.sharding)
            for (s, d) in self.zero_shapes
        ]

    def run(self, in_dev):
        out = self.fn(*in_dev, *self.zeros())
        self.jax.block_until_ready(out)
        return out

    def results(self, out_arrs):
        res = []
        for c in range(NCORES):
            res.append(
                {
                    name: np.asarray(out_arrs[i]).reshape(
                        NCORES, *self.out_avals[i].shape
                    )[c]
                    for i, name in enumerate(self.out_names)
                }
            )
        return res


def _get_runner(repeat=1):
    key = ("runner", repeat)
    if key not in _cache:
        _cache[key] = Runner(repeat)
    return _cache[key]


def _quantize_inputs(x, label, weight):
    """Host-side normalization, fp8 quantization, transposes, label math."""
    import ml_dtypes

    FP8 = ml_dtypes.float8_e4m3

    x = np.asarray(x, dtype=np.float32)
    label = np.asarray(label).astype(np.int64)
    weight = np.asarray(weight, dtype=np.float32)

    xnorm_raw = np.linalg.norm(x.astype(np.float64), axis=1)
    xn = x / xnorm_raw[:, None].astype(np.float32)
    wnorm = np.linalg.norm(weight.astype(np.float64), axis=1)
    wn = weight / wnorm[:, None].astype(np.float32)

    xq = (QSCALE * xn).astype(FP8)              # [B, D]
    wq = (QSCALE * wn).astype(FP8)              # [C, D]

    # xq transposed: [D, B] -> [4, 128, B] -> [128, 4, B]
    xqT = (
        np.ascontiguousarray(xq.T)
        .reshape(4, 128, B)
        .transpose(1, 0, 2)
    )
    xqT = np.ascontiguousarray(xqT)

    in_maps = []
    for c in range(NCORES):
        shard = np.zeros((C_PAD, D), dtype=FP8)
        shard[:C_SH] = wq[c * C_SH : (c + 1) * C_SH]
        wqT = (
            np.ascontiguousarray(shard.T)
            .reshape(4, 128, C_PAD)
            .transpose(1, 0, 2)
        )
        in_maps.append({"xq": xqT, "wq": np.ascontiguousarray(wqT)})

    # ---- host label-side math (fp64) ----
    x_norm = np.clip(xnorm_raw, N_L, N_U)
    ada_margin = (M_U - M_L) / (N_U - N_L) * (x_norm - N_L) + M_L
    cos_m = np.cos(ada_margin)
    sin_m = np.sin(ada_margin)
    th = np.cos(math.pi - ada_margin)
    mm = np.sin(math.pi - ada_margin) * ada_margin

    wl = weight[label]
    wln = wl / np.linalg.norm(wl.astype(np.float64), axis=1)[:, None]
    cos_l = np.einsum("bd,bd->b", xn.astype(np.float64), wln)
    sin_l = np.sqrt(np.maximum(1.0 - cos_l * cos_l, 0.0))
    phi = cos_l * cos_m - sin_l * sin_m
    phi = np.where(cos_l - th > 0, phi, cos_l - mm)

    loss_g_mean = np.mean(x_norm / (N_U * N_U) + 1.0 / x_norm)

    # quantized label cosine — matches the device's label-column term
    cos_l_q = (
        np.einsum(
            "bd,bd->b",
            xq.astype(np.float32),
            wq[label].astype(np.float32),
        ).astype(np.float64)
        / PSUM_SCALE
    )

    host = {
        "phi": phi,
        "cos_l_q": cos_l_q,
        "loss_g_mean": loss_g_mean,
    }
    return in_maps, host


def _combine(results, host=None):
    if host is None:
        host = _cache["host"]
    # sums: [128, NB*NGROUPS], col idx = bt*NGROUPS + g; sample = bt*128 + p
    sums = np.stack(
        [np.asarray(r["sums"], dtype=np.float64) for r in results]
    )  # [cores, 128, NB*NG]
    maxacc = np.stack(
        [np.asarray(r["maxacc"]).astype(np.float32) for r in results]
    )  # [cores, 128, NB, 2048]

    ncores = sums.shape[0]
    sums = sums.reshape(ncores, 128, NB, NGROUPS)

    # per-sample total sum-exp: sum over cores and groups; subtract pads
    persample = sums.sum(axis=(0, 3))            # [128, NB]
    persample = persample.T.reshape(B)           # sample = bt*128 + p
    persample = persample - ncores * float(N_PAD)

    maxexp = maxacc.max(axis=(0, 3))             # [128, NB]
    maxexp = maxexp.T.reshape(B).astype(np.float64)

    phi = host["phi"]
    cos_l_q = host["cos_l_q"]

    corrected = persample - np.exp(S * cos_l_q) + np.exp(S * phi)
    ce = np.mean(np.log(corrected) - S * phi)
    total = ce + LAMBDA_G * host["loss_g_mean"]

    maxcos = np.log(maxexp) / S
    prec1 = 100.0 * np.mean(phi > maxcos)
    return np.float32(total), np.float32(prec1)


def _make_in_maps(x, label, weight):
    """Kept for test.py's profiling path: returns the device input maps."""
    in_maps, host = _quantize_inputs(x, label, weight)
    _cache["host"] = host
    return in_maps


def kernel(x, label, weight):
    runner = _get_runner(1)
    in_maps, host = _quantize_inputs(x, label, weight)
    _cache["host"] = host
    in_dev = runner.put_inputs(in_maps)
    out = runner.run(in_dev)
    return _combine(runner.results(out), host)


# revision 8
# speedup vs baseline: 1.0033x; 1.0033x over previous
"""Trainium2 kernel for MagFace/AdaCos-style margin softmax-CE loss.

Strategy (8 cores, class-parallel, fp8 DoubleRow):
  - Host pre-normalizes x and W rows (fp32), scales by 16, quantizes to
    fp8 e4m3, and pre-transposes both into [128d, k, cols] layouts.
    Classes are sharded across 8 cores (12500 each, padded to 12544).
  - Device per core: for each class-group (6 x 2048 + 1 x 256 cols),
    fp8 DoubleRow matmuls (256-deep contraction, 0.5 cyc/col) compute
    256*cos into a 4-bank PSUM mega-tile per b-tile; one giant ScalarE
    Exp instruction (scale=S/256) produces exp(S*cos) in bf16 and the
    per-sample class-sum via accum_out; DVE reduce_max tracks the
    per-sample max of exp(S*cos).
  - Pad classes have zero weight -> cos 0 -> exp contributes exactly
    1.0, subtracted on host.
  - Host does all the label-column margin math (phi, loss_g) in fp64,
    corrects the sum-exp for the label column (using the quantized
    label cosine so the correction matches the device's own term),
    and combines: CE + lambda_g * g-loss and top-1 accuracy.
"""

import math
import sys

sys.path.insert(0, "/opt/trn_rl_repo")
sys.path.insert(0, "/opt/trn_rl_repo/concourse")

import numpy as np

# ---- problem constants ----
B = 512
D = 512
C = 100000
NCORES = 8
C_SH = C // NCORES          # 12500
C_PAD = 12544               # 24.5 chunks of 512
N_PAD = C_PAD - C_SH        # 44 zero-pad classes per core
S = 30.0
N_U = 110.0
N_L = 10.0
M_U = 1.0
M_L = 0.1
LAMBDA_G = 35.0
QSCALE = 16.0               # fp8 quantization scale for unit-norm rows
PSUM_SCALE = QSCALE * QSCALE  # matmul result = PSUM_SCALE * cos

# class-group column widths per b-tile: 6 * 2048 + 256 = 12544
GROUP_W = [2048] * 6 + [256]
NGROUPS = len(GROUP_W)
NB = 4                      # b-tiles of 128 samples

_cache = {}


def _emit_body(nc, tc, tensors, mybir, bass):
    F32 = mybir.dt.float32
    BF16 = mybir.dt.bfloat16
    FP8 = mybir.dt.float8e4
    ACT = mybir.ActivationFunctionType
    ALU = mybir.AluOpType
    PM = mybir.MatmulPerfMode

    xq_dram = tensors["xq"]
    wq_dram = tensors["wq"]
    sums_dram = tensors["sums"]
    maxacc_dram = tensors["maxacc"]
    wq_ap = wq_dram.ap()

    with (
        tc.tile_pool(name="persist", bufs=1) as pp,
        tc.tile_pool(name="maxp", bufs=2) as max_pool,
        tc.tile_pool(name="expp", bufs=5) as exp_pool,
        tc.tile_pool(name="psum", bufs=2, space=bass.MemorySpace.PSUM) as psum_pool,
    ):
        xq_sb = pp.tile([128, 4, B], FP8)
        nc.sync.dma_start(xq_sb[:], xq_dram.ap())

        # early activation-table load: dummy exp on a memset tile
        warm_sb = pp.tile([128, 8], F32)
        nc.gpsimd.memset(warm_sb[:], 0.0)
        warm_out = pp.tile([128, 8], BF16)
        nc.scalar.activation(warm_out[:], warm_sb[:], ACT.Exp)

        # whole weight shard stays resident in SBUF (49 KiB/partition);
        # small leading chunks so the first matmuls start ASAP
        wq_sb = pp.tile([128, 4, C_PAD], FP8)
        bounds = [0, 512, 1024]
        while bounds[-1] < C_PAD:
            bounds.append(min(bounds[-1] + 1024, C_PAD))
        for c0, c1 in zip(bounds[:-1], bounds[1:]):
            nc.sync.dma_start(wq_sb[:, :, c0:c1], wq_ap[:, :, c0:c1])

        sums_sb = pp.tile([128, NGROUPS * NB], F32)

        for bt in range(NB):
            maxacc = max_pool.tile([128, 2048], BF16, tag="maxacc")
            col = 0
            for g in range(NGROUPS):
                W = GROUP_W[g]
                ps = psum_pool.tile([128, 2048], F32, tag="ps")
                for kp in range(2):
                    lhsT = xq_sb[:, 2 * kp : 2 * kp + 2, bt * 128 : (bt + 1) * 128]
                    nj = max(W // 512, 1)
                    for j in range(nj):
                        w0 = j * 512
                        w1 = min(w0 + 512, W)
                        nc.tensor.matmul(
                            ps[:, w0:w1],
                            lhsT,
                            wq_sb[:, 2 * kp : 2 * kp + 2, col + w0 : col + w1],
                            start=(kp == 0),
                            stop=(kp == 1),
                            perf_mode=PM.DoubleRow,
                            skip_group_check=True,
                        )
                idx = bt * NGROUPS + g
                exp_t = exp_pool.tile([128, 2048], BF16, tag="exp")
                nc.scalar.activation(
                    exp_t[:, :W], ps[:, :W], ACT.Exp,
                    scale=float(S / PSUM_SCALE),
                    accum_out=sums_sb[:, idx : idx + 1],
                )
                if g == 0:
                    nc.vector.tensor_copy(maxacc[:], exp_t[:])
                else:
                    nc.vector.tensor_tensor(
                        out=maxacc[:, :W], in0=maxacc[:, :W],
                        in1=exp_t[:, :W], op=ALU.max,
                    )
                col += W
            nc.sync.dma_start(
                maxacc_dram.ap()[:, bt, :], maxacc[:]
            )

        nc.sync.dma_start(sums_dram.ap(), sums_sb[:])


def _build(repeat=1):
    from concourse import bass, bacc, tile, mybir

    F32 = mybir.dt.float32
    FP8 = mybir.dt.float8e4

    nc = bacc.Bacc("TRN2", target_bir_lowering=False, debug=False)

    tensors = {
        "xq": nc.dram_tensor("xq", [128, 4, B], FP8, kind="ExternalInput"),
        "wq": nc.dram_tensor("wq", [128, 4, C_PAD], FP8, kind="ExternalInput"),
        "sums": nc.dram_tensor(
            "sums", [128, NGROUPS * NB], F32, kind="ExternalOutput"
        ),
        "maxacc": nc.dram_tensor(
            "maxacc", [128, NB, 2048], mybir.dt.bfloat16, kind="ExternalOutput"
        ),
    }

    with tile.TileContext(nc) as tc:
        for _ in range(repeat):
            _emit_body(nc, tc, tensors, mybir, bass)

    nc.compile()
    return nc


class Runner:
    """Persistent jitted 8-core runner (inputs stay device-resident)."""

    def __init__(self, repeat=1):
        import jax
        from jax.sharding import Mesh, PartitionSpec, NamedSharding
        from jax.experimental.shard_map import shard_map
        from concourse import bass2jax, mybir

        self.jax = jax
        nc = _build(repeat)
        self.nc = nc
        bass2jax.install_neuronx_cc_hook()

        partition_name = (
            nc.partition_id_tensor.name if nc.partition_id_tensor else None
        )
        in_names, out_names, out_avals, zero_shapes = [], [], [], []
        for alloc in nc.m.functions[0].allocations:
            if not isinstance(alloc, mybir.MemoryLocationSet):
                continue
            name = alloc.memorylocations[0].name
            if alloc.kind == "ExternalInput":
                if name == partition_name:
                    continue
                in_names.append(name)
            elif alloc.kind == "ExternalOutput":
                shape = tuple(alloc.tensor_shape)
                dtype = mybir.dt.np(alloc.dtype)
                out_names.append(name)
                out_avals.append(jax.core.ShapedArray(shape, dtype))
                zero_shapes.append((shape, dtype))
        self.in_names = in_names
        self.out_names = out_names
        self.out_avals = out_avals
        self.zero_shapes = zero_shapes
        n_params = len(in_names)
        n_outs = len(out_names)
        all_in_names = in_names + out_names
        if partition_name is not None:
            all_in_names = all_in_names + [partition_name]

        def _body(*args):
            operands = list(args)
            if partition_name is not None:
                operands.append(bass2jax.partition_id_tensor())
            outs = bass2jax._bass_exec_p.bind(
                *operands,
                out_avals=tuple(out_avals),
                in_names=tuple(all_in_names),
                out_names=tuple(out_names),
                lowering_input_output_aliases=(),
                sim_require_finite=True,
                sim_require_nnan=True,
                nc=nc,
            )
            return tuple(outs)

        devices = jax.devices()[:NCORES]
        self.mesh = Mesh(np.asarray(devices), ("core",))
        in_specs = (PartitionSpec("core"),) * (n_params + n_outs)
        out_specs = (PartitionSpec("core"),) * n_outs
        self.sharding = NamedSharding(self.mesh, PartitionSpec("core"))
        self.fn = jax.jit(
            shard_map(
                _body, mesh=self.mesh, in_specs=in_specs, out_specs=out_specs,
                check_rep=False,
            ),
            donate_argnums=tuple(range(n_params, n_params + n_outs)),
            keep_unused=True,
        )

    def put_inputs(self, in_maps):
        jax = self.jax
        concat = [
            np.concatenate([np.asarray(m[name]) for m in in_maps], axis=0)
            for name in self.in_names
        ]
        return [jax.device_put(a, self.sharding) for a in concat]

    def zeros(self):
        jax = self.jax
        return [
            jax.device_put(np.zeros((NCORES * s[0], *s[1:]), d), self.sharding)
            for (s, d) in self.zero_shapes
        ]

    def run(self, in_dev):
        out = self.fn(*in_dev, *self.zeros())
        self.jax.block_until_ready(out)
        return out

    def results(self, out_arrs):
        res = []
        for c in range(NCORES):
            res.append(
                {
                    name: np.asarray(out_arrs[i]).reshape(
                        NCORES, *self.out_avals[i].shape
                    )[c]
                    for i, name in enumerate(self.out_names)
                }
            )
        return res


def _get_runner(repeat=1):
    key = ("runner", repeat)
    if key not in _cache:
        _cache[key] = Runner(repeat)
    return _cache[key]


def _quantize_inputs(x, label, weight):
    """Host-side normalization, fp8 quantization, transposes, label math."""
    import ml_dtypes

    FP8 = ml_dtypes.float8_e4m3

    x = np.asarray(x, dtype=np.float32)
    label = np.asarray(label).astype(np.int64)
    weight = np.asarray(weight, dtype=np.float32)

    xnorm_raw = np.linalg.norm(x.astype(np.float64), axis=1)
    xn = x / xnorm_raw[:, None].astype(np.float32)
    wnorm = np.linalg.norm(weight.astype(np.float64), axis=1)
    wn = weight / wnorm[:, None].astype(np.float32)

    xq = (QSCALE * xn).astype(FP8)              # [B, D]
    wq = (QSCALE * wn).astype(FP8)              # [C, D]

    # xq transposed: [D, B] -> [4, 128, B] -> [128, 4, B]
    xqT = (
        np.ascontiguousarray(xq.T)
        .reshape(4, 128, B)
        .transpose(1, 0, 2)
    )
    xqT = np.ascontiguousarray(xqT)

    in_maps = []
    for c in range(NCORES):
        shard = np.zeros((C_PAD, D), dtype=FP8)
        shard[:C_SH] = wq[c * C_SH : (c + 1) * C_SH]
        wqT = (
            np.ascontiguousarray(shard.T)
            .reshape(4, 128, C_PAD)
            .transpose(1, 0, 2)
        )
        in_maps.append({"xq": xqT, "wq": np.ascontiguousarray(wqT)})

    # ---- host label-side math (fp64) ----
    x_norm = np.clip(xnorm_raw, N_L, N_U)
    ada_margin = (M_U - M_L) / (N_U - N_L) * (x_norm - N_L) + M_L
    cos_m = np.cos(ada_margin)
    sin_m = np.sin(ada_margin)
    th = np.cos(math.pi - ada_margin)
    mm = np.sin(math.pi - ada_margin) * ada_margin

    wl = weight[label]
    wln = wl / np.linalg.norm(wl.astype(np.float64), axis=1)[:, None]
    cos_l = np.einsum("bd,bd->b", xn.astype(np.float64), wln)
    sin_l = np.sqrt(np.maximum(1.0 - cos_l * cos_l, 0.0))
    phi = cos_l * cos_m - sin_l * sin_m
    phi = np.where(cos_l - th > 0, phi, cos_l - mm)

    loss_g_mean = np.mean(x_norm / (N_U * N_U) + 1.0 / x_norm)

    # quantized label cosine — matches the device's label-column term
    cos_l_q = (
        np.einsum(
            "bd,bd->b",
            xq.astype(np.float32),
            wq[label].astype(np.float32),
        ).astype(np.float64)
        / PSUM_SCALE
    )

    host = {
        "phi": phi,
        "cos_l_q": cos_l_q,
        "loss_g_mean": loss_g_mean,
    }
    return in_maps, host


def _combine(results, host=None):
    if host is None:
        host = _cache["host"]
    # sums: [128, NB*NGROUPS], col idx = bt*NGROUPS + g; sample = bt*128 + p
    sums = np.stack(
        [np.asarray(r["sums"], dtype=np.float64) for r in results]
    )  # [cores, 128, NB*NG]
    maxacc = np.stack(
        [np.asarray(r["maxacc"]).astype(np.float32) for r in results]
    )  # [cores, 128, NB, 2048]

    ncores = sums.shape[0]
    sums = sums.reshape(ncores, 128, NB, NGROUPS)

    # per-sample total sum-exp: sum over cores and groups; subtract pads
    persample = sums.sum(axis=(0, 3))            # [128, NB]
    persample = persample.T.reshape(B)           # sample = bt*128 + p
    persample = persample - ncores * float(N_PAD)

    maxexp = maxacc.max(axis=(0, 3))             # [128, NB]
    maxexp = maxexp.T.reshape(B).astype(np.float64)

    phi = host["phi"]
    cos_l_q = host["cos_l_q"]

    corrected = persample - np.exp(S * cos_l_q) + np.exp(S * phi)
    ce = np.mean(np.log(corrected) - S * phi)
    total = ce + LAMBDA_G * host["loss_g_mean"]

    maxcos = np.log(maxexp) / S
    prec1 = 100.0 * np.mean(phi > maxcos)
    return np.float32(total), np.float32(prec1)


def _make_in_maps(x, label, weight):
    """Kept for test.py's profiling path: returns the device input maps."""
    in_maps, host = _quantize_inputs(x, label, weight)
    _cache["host"] = host
    return in_maps


def kernel(x, label, weight):
    runner = _get_runner(1)
    in_maps, host = _quantize_inputs(x, label, weight)
    _cache["host"] = host
    in_dev = runner.put_inputs(in_maps)
    out = runner.run(in_dev)
    return _combine(runner.results(out), host)


# revision 10
# speedup vs baseline: 1.1643x; 1.1605x over previous
"""Trainium2 kernel for MagFace/AdaCos-style margin softmax-CE loss.

Strategy (8 cores, class-parallel, fp8 DoubleRow):
  - Host pre-normalizes x and W rows (fp32), scales by 16, quantizes to
    fp8 e4m3, and pre-transposes both into [128d, k, cols] layouts.
    Classes are sharded across 8 cores (12500 each, padded to 12544).
  - Device per core: for each class-group (6 x 2048 + 1 x 256 cols),
    fp8 DoubleRow matmuls (256-deep contraction, 0.5 cyc/col) compute
    256*cos into a 4-bank PSUM mega-tile per b-tile; one giant ScalarE
    Exp instruction (scale=S/256) produces exp(S*cos) in bf16 and the
    per-sample class-sum via accum_out; DVE reduce_max tracks the
    per-sample max of exp(S*cos).
  - Pad classes have zero weight -> cos 0 -> exp contributes exactly
    1.0, subtracted on host.
  - Host does all the label-column margin math (phi, loss_g) in fp64,
    corrects the sum-exp for the label column (using the quantized
    label cosine so the correction matches the device's own term),
    and combines: CE + lambda_g * g-loss and top-1 accuracy.
"""

import math
import sys

sys.path.insert(0, "/opt/trn_rl_repo")
sys.path.insert(0, "/opt/trn_rl_repo/concourse")

import numpy as np

# ---- problem constants ----
B = 512
D = 512
C = 100000
NCORES = 8
C_SH = C // NCORES          # 12500
C_PAD = 12544               # 24.5 chunks of 512
N_PAD = C_PAD - C_SH        # 44 zero-pad classes per core
S = 30.0
N_U = 110.0
N_L = 10.0
M_U = 1.0
M_L = 0.1
LAMBDA_G = 35.0
QSCALE = 16.0               # fp8 quantization scale for unit-norm rows
PSUM_SCALE = QSCALE * QSCALE  # matmul result = PSUM_SCALE * cos

# class-group column widths per b-tile: 6 * 2048 + 256 = 12544
GROUP_W = [2048] * 6 + [256]
NGROUPS = len(GROUP_W)
NB = 4                      # b-tiles of 128 samples

_cache = {}


def _emit_body(nc, tc, tensors, mybir, bass):
    F32 = mybir.dt.float32
    BF16 = mybir.dt.bfloat16
    FP8 = mybir.dt.float8e4
    ACT = mybir.ActivationFunctionType
    ALU = mybir.AluOpType
    PM = mybir.MatmulPerfMode

    xq_dram = tensors["xq"]
    wq_dram = tensors["wq"]
    sums_dram = tensors["sums"]
    maxacc_dram = tensors["maxacc"]
    wq_ap = wq_dram.ap()

    with (
        tc.tile_pool(name="persist", bufs=1) as pp,
        tc.tile_pool(name="expp", bufs=5) as exp_pool,
        tc.tile_pool(name="psum", bufs=2, space=bass.MemorySpace.PSUM) as psum_pool,
    ):
        xq_sb = pp.tile([128, 4, B], FP8)
        nc.sync.dma_start(xq_sb[:], xq_dram.ap())

        # early activation-table load: dummy exp on a memset tile
        warm_sb = pp.tile([128, 8], F32)
        nc.gpsimd.memset(warm_sb[:], 0.0)
        warm_out = pp.tile([128, 8], BF16)
        nc.scalar.activation(warm_out[:], warm_sb[:], ACT.Exp)

        # running per-btile elementwise max of exp values, zero-initialized
        # (exp > 0 always wins); gpsimd memset runs during the DMA ramp
        maxacc = pp.tile([128, NB, 2048], BF16)
        nc.gpsimd.memset(maxacc[:], 0.0)

        # whole weight shard stays resident in SBUF (49 KiB/partition).
        # The small class-group (cols 12288:12544) is processed FIRST, so
        # its chunk is DMA'd first and the first (cheap) exps start ASAP.
        wq_sb = pp.tile([128, 4, C_PAD], FP8)
        bounds = [(12288, C_PAD), (0, 512), (512, 1024)]
        c0 = 1024
        while c0 < 12288:
            bounds.append((c0, min(c0 + 1024, 12288)))
            c0 += 1024
        for a, b in bounds:
            nc.sync.dma_start(wq_sb[:, :, a:b], wq_ap[:, :, a:b])

        sums_sb = pp.tile([128, NGROUPS * NB], F32)

        # group order: small group first, then the six 2048-wide groups
        order = [(12288, 256)] + [(i * 2048, 2048) for i in range(6)]
        for gi, (col, W) in enumerate(order):
            g = NGROUPS - 1 if W == 256 else col // 2048
            for bt in range(NB):
                ps = psum_pool.tile([128, 2048], F32, tag="ps")
                for kp in range(2):
                    lhsT = xq_sb[:, 2 * kp : 2 * kp + 2, bt * 128 : (bt + 1) * 128]
                    for j in range(max(W // 512, 1)):
                        w0 = j * 512
                        w1 = min(w0 + 512, W)
                        nc.tensor.matmul(
                            ps[:, w0:w1],
                            lhsT,
                            wq_sb[:, 2 * kp : 2 * kp + 2, col + w0 : col + w1],
                            start=(kp == 0),
                            stop=(kp == 1),
                            perf_mode=PM.DoubleRow,
                            skip_group_check=True,
                        )
                idx = bt * NGROUPS + g
                exp_t = exp_pool.tile([128, 2048], BF16, tag="exp")
                nc.scalar.activation(
                    exp_t[:, :W], ps[:, :W], ACT.Exp,
                    scale=float(S / PSUM_SCALE),
                    accum_out=sums_sb[:, idx : idx + 1],
                )
                nc.vector.tensor_tensor(
                    out=maxacc[:, bt, :W], in0=maxacc[:, bt, :W],
                    in1=exp_t[:, :W], op=ALU.max,
                )
                if gi == len(order) - 1:
                    nc.sync.dma_start(
                        maxacc_dram.ap()[:, bt, :], maxacc[:, bt, :]
                    )

        nc.sync.dma_start(sums_dram.ap(), sums_sb[:])


def _build(repeat=1):
    from concourse import bass, bacc, tile, mybir

    F32 = mybir.dt.float32
    FP8 = mybir.dt.float8e4

    nc = bacc.Bacc("TRN2", target_bir_lowering=False, debug=False)

    tensors = {
        "xq": nc.dram_tensor("xq", [128, 4, B], FP8, kind="ExternalInput"),
        "wq": nc.dram_tensor("wq", [128, 4, C_PAD], FP8, kind="ExternalInput"),
        "sums": nc.dram_tensor(
            "sums", [128, NGROUPS * NB], F32, kind="ExternalOutput"
        ),
        "maxacc": nc.dram_tensor(
            "maxacc", [128, NB, 2048], mybir.dt.bfloat16, kind="ExternalOutput"
        ),
    }

    with tile.TileContext(nc) as tc:
        for _ in range(repeat):
            _emit_body(nc, tc, tensors, mybir, bass)

    nc.compile()
    return nc


class Runner:
    """Persistent jitted 8-core runner (inputs stay device-resident)."""

    def __init__(self, repeat=1):
        import jax
        from jax.sharding import Mesh, PartitionSpec, NamedSharding
        from jax.experimental.shard_map import shard_map
        from concourse import bass2jax, mybir

        self.jax = jax
        nc = _build(repeat)
        self.nc = nc
        bass2jax.install_neuronx_cc_hook()

        partition_name = (
            nc.partition_id_tensor.name if nc.partition_id_tensor else None
        )
        in_names, out_names, out_avals, zero_shapes = [], [], [], []
        for alloc in nc.m.functions[0].allocations:
            if not isinstance(alloc, mybir.MemoryLocationSet):
                continue
            name = alloc.memorylocations[0].name
            if alloc.kind == "ExternalInput":
                if name == partition_name:
                    continue
                in_names.append(name)
            elif alloc.kind == "ExternalOutput":
                shape = tuple(alloc.tensor_shape)
                dtype = mybir.dt.np(alloc.dtype)
                out_names.append(name)
                out_avals.append(jax.core.ShapedArray(shape, dtype))
                zero_shapes.append((shape, dtype))
        self.in_names = in_names
        self.out_names = out_names
        self.out_avals = out_avals
        self.zero_shapes = zero_shapes
        n_params = len(in_names)
        n_outs = len(out_names)
        all_in_names = in_names + out_names
        if partition_name is not None:
            all_in_names = all_in_names + [partition_name]

        def _body(*args):
            operands = list(args)
            if partition_name is not None:
                operands.append(bass2jax.partition_id_tensor())
            outs = bass2jax._bass_exec_p.bind(
                *operands,
                out_avals=tuple(out_avals),
                in_names=tuple(all_in_names),
                out_names=tuple(out_names),
                lowering_input_output_aliases=(),
                sim_require_finite=True,
                sim_require_nnan=True,
                nc=nc,
            )
            return tuple(outs)

        devices = jax.devices()[:NCORES]
        self.mesh = Mesh(np.asarray(devices), ("core",))
        in_specs = (PartitionSpec("core"),) * (n_params + n_outs)
        out_specs = (PartitionSpec("core"),) * n_outs
        self.sharding = NamedSharding(self.mesh, PartitionSpec("core"))
        self.fn = jax.jit(
            shard_map(
                _body, mesh=self.mesh, in_specs=in_specs, out_specs=out_specs,
                check_rep=False,
            ),
            donate_argnums=tuple(range(n_params, n_params + n_outs)),
            keep_unused=True,
        )

    def put_inputs(self, in_maps):
        jax = self.jax
        concat = [
            np.concatenate([np.asarray(m[name]) for m in in_maps], axis=0)
            for name in self.in_names
        ]
        return [jax.device_put(a, self.sharding) for a in concat]

    def zeros(self):
        jax = self.jax
        return [
            jax.device_put(np.zeros((NCORES * s[0], *s[1:]), d), self.sharding)
            for (s, d) in self.zero_shapes
        ]

    def run(self, in_dev):
        out = self.fn(*in_dev, *self.zeros())
        self.jax.block_until_ready(out)
        return out

    def results(self, out_arrs):
        res = []
        for c in range(NCORES):
            res.append(
                {
                    name: np.asarray(out_arrs[i]).reshape(
                        NCORES, *self.out_avals[i].shape
                    )[c]
                    for i, name in enumerate(self.out_names)
                }
            )
        return res


def _get_runner(repeat=1):
    key = ("runner", repeat)
    if key not in _cache:
        _cache[key] = Runner(repeat)
    return _cache[key]


def _quantize_inputs(x, label, weight):
    """Host-side normalization, fp8 quantization, transposes, label math."""
    import ml_dtypes

    FP8 = ml_dtypes.float8_e4m3

    x = np.asarray(x, dtype=np.float32)
    label = np.asarray(label).astype(np.int64)
    weight = np.asarray(weight, dtype=np.float32)

    xnorm_raw = np.linalg.norm(x.astype(np.float64), axis=1)
    xn = x / xnorm_raw[:, None].astype(np.float32)
    wnorm = np.linalg.norm(weight.astype(np.float64), axis=1)
    wn = weight / wnorm[:, None].astype(np.float32)

    xq = (QSCALE * xn).astype(FP8)              # [B, D]
    wq = (QSCALE * wn).astype(FP8)              # [C, D]

    # xq transposed: [D, B] -> [4, 128, B] -> [128, 4, B]
    xqT = (
        np.ascontiguousarray(xq.T)
        .reshape(4, 128, B)
        .transpose(1, 0, 2)
    )
    xqT = np.ascontiguousarray(xqT)

    in_maps = []
    for c in range(NCORES):
        shard = np.zeros((C_PAD, D), dtype=FP8)
        shard[:C_SH] = wq[c * C_SH : (c + 1) * C_SH]
        wqT = (
            np.ascontiguousarray(shard.T)
            .reshape(4, 128, C_PAD)
            .transpose(1, 0, 2)
        )
        in_maps.append({"xq": xqT, "wq": np.ascontiguousarray(wqT)})

    # ---- host label-side math (fp64) ----
    x_norm = np.clip(xnorm_raw, N_L, N_U)
    ada_margin = (M_U - M_L) / (N_U - N_L) * (x_norm - N_L) + M_L
    cos_m = np.cos(ada_margin)
    sin_m = np.sin(ada_margin)
    th = np.cos(math.pi - ada_margin)
    mm = np.sin(math.pi - ada_margin) * ada_margin

    wl = weight[label]
    wln = wl / np.linalg.norm(wl.astype(np.float64), axis=1)[:, None]
    cos_l = np.einsum("bd,bd->b", xn.astype(np.float64), wln)
    sin_l = np.sqrt(np.maximum(1.0 - cos_l * cos_l, 0.0))
    phi = cos_l * cos_m - sin_l * sin_m
    phi = np.where(cos_l - th > 0, phi, cos_l - mm)

    loss_g_mean = np.mean(x_norm / (N_U * N_U) + 1.0 / x_norm)

    # quantized label cosine — matches the device's label-column term
    cos_l_q = (
        np.einsum(
            "bd,bd->b",
            xq.astype(np.float32),
            wq[label].astype(np.float32),
        ).astype(np.float64)
        / PSUM_SCALE
    )

    host = {
        "phi": phi,
        "cos_l_q": cos_l_q,
        "loss_g_mean": loss_g_mean,
    }
    return in_maps, host


def _combine(results, host=None):
    if host is None:
        host = _cache["host"]
    # sums: [128, NB*NGROUPS], col idx = bt*NGROUPS + g; sample = bt*128 + p
    sums = np.stack(
        [np.asarray(r["sums"], dtype=np.float64) for r in results]
    )  # [cores, 128, NB*NG]
    maxacc = np.stack(
        [np.asarray(r["maxacc"]).astype(np.float32) for r in results]
    )  # [cores, 128, NB, 2048]

    ncores = sums.shape[0]
    sums = sums.reshape(ncores, 128, NB, NGROUPS)

    # per-sample total sum-exp: sum over cores and groups; subtract pads
    persample = sums.sum(axis=(0, 3))            # [128, NB]
    persample = persample.T.reshape(B)           # sample = bt*128 + p
    persample = persample - ncores * float(N_PAD)

    maxexp = maxacc.max(axis=(0, 3))             # [128, NB]
    maxexp = maxexp.T.reshape(B).astype(np.float64)

    phi = host["phi"]
    cos_l_q = host["cos_l_q"]

    corrected = persample - np.exp(S * cos_l_q) + np.exp(S * phi)
    ce = np.mean(np.log(corrected) - S * phi)
    total = ce + LAMBDA_G * host["loss_g_mean"]

    maxcos = np.log(maxexp) / S
    prec1 = 100.0 * np.mean(phi > maxcos)
    return np.float32(total), np.float32(prec1)


def _make_in_maps(x, label, weight):
    """Kept for test.py's profiling path: returns the device input maps."""
    in_maps, host = _quantize_inputs(x, label, weight)
    _cache["host"] = host
    return in_maps


def kernel(x, label, weight):
    runner = _get_runner(1)
    in_maps, host = _quantize_inputs(x, label, weight)
    _cache["host"] = host
    in_dev = runner.put_inputs(in_maps)
    out = runner.run(in_dev)
    return _combine(runner.results(out), host)
